# revision 34
# baseline (speedup 1.0000x reference)
"""CapsuleLayer (dynamic routing) Trainium2 Bass kernel, v2.

Sharding: pure data-parallel over batch B=256 -> 8 cores x 32 batches.
Per core the 32 batches run as 4 sub-chunks of 8; SBUF partition dim packs
p = b*16 + ig where capsule index i = 16*g + ig, g in [0,72).

Phase 1 (u_hat): K=8 contraction packed to K=128 by block-diagonalizing 16
capsules' inputs into the PE stationary operand (staged zero-padded on the
host). Matmuls write 6 g's per 2-bank PSUM tile; f32->bf16 evacuation
copies rotate DVE/ACT/Pool. u_hat stays on-chip as U[128, G, D, N] bf16.

r0 shortcut: sum_i u_hat is computed DIRECTLY from a dense x2 stationary
([128,(ig,k)] x [32,(s,b)]) against the same W2 moving operand, PSUM-
accumulated over all 72 g. It needs no U evacuation, so round-0 squash +
agreement for sub s start while later subs are still in phase 1.

Routing r1/r2 stream per sub-chunk: softmax -> tm = c (.) U (DVE) ->
bm32-masked PE partition-reduce accumulated F=160 into a per-sub PSUM
region (no MMB reduce needed) -> squash on the 32-row region (invalid rows
are zeros; harmless) -> bc32 PE broadcast back to (b,ig) partitions.
Agreement folds d 16->1 via an add tree: mul/L1/L2 on DVE (bf16 2x mode),
L3/L4/bl on Pool via scalar_tensor_tensor (0.60 efficiency class vs 0.42
for TensorTensor).
"""

import numpy as np
import ml_dtypes
import os

B, N, I, D, DK = 256, 10, 1152, 16, 8
NCORES = 8
BC = B // NCORES      # 32 batches per core
BS = 8                # batches per sub-chunk
NSUB = BC // BS       # 4
IG = 16               # capsules per PE group
G = I // IG           # 72
ND = D * N            # 160, (d-major, n-minor)
GBLK = 12             # g per routing block
NBLK = G // GBLK      # 6
GCP = 6               # g's per phase-1 PSUM tile (3 per bank x 2 banks)
NCP = G // GCP        # 12 copies per sub-chunk
PREG = 256            # f32 stride between per-sub PSUM po regions (1KB)

# engine knobs (tunable via env for experiments)
UCP = os.environ.get("K_UCP", "daa")   # copy engine cycle d=DVE a=ACT
                                        # (GPSIMD cannot access PSUM)
# Per-sub Pool-assigned g-block counts (of NBLK=6, taken from the tail).
# Pool's only efficient use is whole 1920-elem muls (1.98 ns/elem + 95 ns
# launch); all folds stay on DVE.
AGR0_PN = [int(c) for c in os.environ.get("K_AGR0PN", "3232")]
AGR1_PN = [int(c) for c in os.environ.get("K_AGR1PN", "3232")]
TM_PN = {1: [int(c) for c in os.environ.get("K_TM1PN", "1111")],
         2: [int(c) for c in os.environ.get("K_TM2PN", "1111")]}
BF16 = ml_dtypes.bfloat16

_cache = {}


def _bcast(ap, axis, count):
    ap = ap.unsqueeze(axis)
    shape = list(ap.shape)
    shape[axis] = count
    return ap.broadcast_to(shape)


def _legalize_waits(nc):
    """This walrus build takes at most 1 embedded sync wait per TPB
    instruction (2 on EventSemaphore, 0 on Drain). Tile emits multi-wait
    sync_info; hoist the extras onto preceding EventSemaphore instructions
    on the same engine queue."""
    from concourse import mybir

    n = 0
    for fn in nc.m.functions:
        for blk in fn.blocks:
            out = []
            for inst in blk.instructions:
                si = inst.sync_info
                if si is not None and si.on_wait:
                    keep = 1
                    if inst.opcode == "Drain":
                        keep = 0
                    elif inst.opcode == "EventSemaphore":
                        keep = 2
                    w = list(si.on_wait)
                    if len(w) > keep:
                        extra = w[:len(w) - keep] if keep else w
                        kept = w[len(w) - keep:] if keep else []
                        for i0 in range(0, len(extra), 2):
                            n += 1
                            out.append(mybir.InstEventSemaphore(
                                name=f"{inst.name}-hw{n}",
                                engine=inst.engine, ins=[], outs=[],
                                sync_info=mybir.SyncInfo(
                                    on_wait=extra[i0:i0 + 2],
                                    on_update=[]),
                            ))
                        si.on_wait = kept
                out.append(inst)
            blk.instructions = out
    return n


def _build_nc():
    import concourse.bass as bass
    import concourse.tile as tile
    from concourse import mybir
    from contextlib import ExitStack

    f32 = mybir.dt.float32
    bf16 = mybir.dt.bfloat16
    AX = mybir.AxisListType
    OP = mybir.AluOpType
    AF = mybir.ActivationFunctionType

    def _view(ap, off, dims):
        return bass.AP(tensor=ap.tensor, offset=ap.offset + off,
                       ap=[list(ap.ap[0])] + [list(d) for d in dims])

    nc = bass.Bass()
    xb_d = nc.dram_tensor("xblk", [128, NSUB, G * BS * IG], bf16,
                          kind="ExternalInput")
    x2_d = nc.dram_tensor("x2", [128, G * BC], bf16, kind="ExternalInput")
    w2_d = nc.dram_tensor("w2", [128, G * ND], bf16, kind="ExternalInput")
    bm32_d = nc.dram_tensor("bmask32", [128, NSUB, NSUB * BS], bf16,
                            kind="ExternalInput")
    bc32_d = nc.dram_tensor("bcmask32", [32, NSUB, 128], bf16,
                            kind="ExternalInput")
    y_d = nc.dram_tensor("y", [NSUB, NSUB * BS, ND], f32,
                         kind="ExternalOutput")
    DBG = os.environ.get("K_DBG", "0") == "1"
    if DBG:
        dbg_v0 = nc.dram_tensor("dbg_v0", [NSUB * BS, ND], f32,
                                kind="ExternalOutput")
        dbg_bl = nc.dram_tensor("dbg_bl", [128, NSUB * G * N], f32,
                                kind="ExternalOutput")
        dbg_u = nc.dram_tensor("dbg_u", [128, G * ND], bf16,
                               kind="ExternalOutput")

    with tile.TileContext(nc) as tc:
        with ExitStack() as ctx:
            singles = ctx.enter_context(tc.tile_pool(name="singles", bufs=1))
            upool = ctx.enter_context(tc.tile_pool(name="upool", bufs=4))
            outps = ctx.enter_context(
                tc.tile_pool(name="outps", bufs=1, space="PSUM"))
            bcps = ctx.enter_context(
                tc.tile_pool(name="bcps", bufs=2, space="PSUM"))

            GQ = G // 4
            Us = []
            # po psum: one [32, PREG] region per sub (r0 uses region 0 for
            # all subs). Regions padded to 1KB so matmul outs stay in-bank.
            ENG = {"d": nc.vector, "a": nc.scalar, "p": nc.gpsimd}

            def ucopy(k, dst, src):
                e = ENG[UCP[k % len(UCP)]]
                if e is nc.scalar:
                    e.copy(dst, src)
                else:
                    e.tensor_copy(dst, src)

            tpool = ctx.enter_context(tc.tile_pool(name="tpool", bufs=2))
            tfpool = ctx.enter_context(tc.tile_pool(name="tfpool", bufs=1))
            blpool = ctx.enter_context(tc.tile_pool(name="blpool", bufs=1))
            smpool = ctx.enter_context(tc.tile_pool(name="smpool", bufs=1))
            obcpool = ctx.enter_context(tc.tile_pool(name="obc", bufs=4))
            tiny = ctx.enter_context(tc.tile_pool(name="tiny", bufs=1))

            bl = blpool.tile([128, NSUB, G, N], f32, tag="bl")

            def squash(po, roff, r, s, alpha, emit_y=False):
                """Squash one [32, D, N] po region (psum view at f32 offset
                roff). Invalid rows are zeros: sqrt(0)=0, 1/(1+0)=1 - safe.
                Returns bf16 ov (or f32 when emit_y, DMA'd to y)."""
                tg = f"{r}{'' if s is None else s}"
                v = _view(po, roff, [[N, D], [1, N]])
                # ACT square reads PSUM once; the alpha^2 scale rides the
                # reduce-free nsq path (folded into fac via alpha in ov)
                vsq = tiny.tile([32, D, N], f32, tag="vsq", name=f"vsq{tg}")
                nc.scalar.square(vsq, v)
                nsq = tiny.tile([32, N], f32, tag="ns", name=f"ns{tg}")
                nc.vector.tensor_reduce(
                    nsq, vsq.transpose([0, 2, 1]), axis=AX.X, op=OP.add)
                if alpha != 1.0:
                    nc.vector.tensor_scalar_mul(nsq, nsq, alpha * alpha)
                # fac = sqrt(nsq)/(1+nsq): exact squash scale, and all-zero
                # (invalid) rows give 0/(1+0) = 0 instead of 0*inf = NaN
                sq = tiny.tile([32, N], f32, tag="sq", name=f"sq{tg}")
                nc.scalar.sqrt(sq, nsq)
                t1 = tiny.tile([32, N], f32, tag="t1", name=f"t1{tg}")
                nc.vector.tensor_scalar_add(t1, nsq, 1.0)
                rec = tiny.tile([32, N], f32, tag="rec", name=f"rec{tg}")
                nc.vector.reciprocal(rec, t1)
                fac = tiny.tile([32, N], f32, tag="fac", name=f"fac{tg}")
                nc.vector.tensor_mul(fac, sq, rec)
                if emit_y:
                    ov = tiny.tile([32, D, N], f32, tag="ovf",
                                   name=f"ovf{tg}")
                    nc.vector.scalar_tensor_tensor(
                        ov, v, alpha, _bcast(fac, 1, D),
                        op0=OP.mult, op1=OP.mult)
                    nc.sync.dma_start(y_d[s], _view(ov, 0, [[1, ND]]))
                    return None
                ov = tiny.tile([32, D, N], bf16, tag=f"ov{0 if s is None else s % 2}",
                               name=f"ov{tg}")
                nc.vector.scalar_tensor_tensor(
                    ov, v, alpha, _bcast(fac, 1, D), op0=OP.mult, op1=OP.mult)
                return ov

            def bcast_out(ov, r, s, eng_d):
                """Broadcast squash output rows (8s..8s+8) to the (b,ig)
                partitions of sub s."""
                psb = bcps.tile([128, D, N], f32, tag="bc", name=f"bc{r}{s}")
                nc.tensor.matmul(psb, bc32[:, s], ov, start=True, stop=True)
                obc = obcpool.tile([128, D, N], bf16, tag="obc",
                                   name=f"obc{r}{s}")
                if eng_d:
                    nc.vector.tensor_copy(obc, psb)
                else:
                    nc.scalar.copy(obc, psb)
                return obc

            def fold_range(s, t2, t2f, ga, gb, first, sfx):
                """DVE d-fold chain 16->1 over g range [ga, gb) into bl."""
                ng = gb - ga
                nc.vector.tensor_add(t2f[:, ga:gb], t2[:, ga:gb, 0:8],
                                     t2[:, ga:gb, 8:16])
                nc.vector.tensor_add(t2f[:, ga:gb, 0:4], t2f[:, ga:gb, 0:4],
                                     t2f[:, ga:gb, 4:8])
                nc.vector.tensor_add(t2f[:, ga:gb, 0:2], t2f[:, ga:gb, 0:2],
                                     t2f[:, ga:gb, 2:4])
                if first:
                    nc.vector.tensor_add(bl[:, s, ga:gb],
                                         t2f[:, ga:gb, 0], t2f[:, ga:gb, 1])
                else:
                    nc.vector.tensor_add(t2f[:, ga:gb, 0], t2f[:, ga:gb, 0],
                                         t2f[:, ga:gb, 1])
                    nc.vector.tensor_add(bl[:, s, ga:gb],
                                         bl[:, s, ga:gb], t2f[:, ga:gb, 0])

            def agr_sub(s, obc, first, tag, pool_n=0):
                """bl[:, s] (+)= sum_d U[s] * obc  (one sub). The last
                pool_n g-blocks' muls run on Pool; DVE runs one merged mul
                over the leading blocks, then two merged fold chains."""
                t2 = tpool.tile([128, G, D, N], bf16, tag="t2",
                                name=f"t2{tag}")
                t2f = tfpool.tile([128, G, 8, N], bf16, tag="t2f",
                                  name=f"t2f{tag}")
                gs = (NBLK - pool_n) * GBLK
                nc.vector.tensor_mul(t2[:, 0:gs], Us[s][:, 0:gs],
                                     _bcast(obc, 1, gs))
                for blk in range(NBLK - pool_n, NBLK):
                    g0 = blk * GBLK
                    nc.gpsimd.tensor_mul(t2[:, g0:g0 + GBLK],
                                         Us[s][:, g0:g0 + GBLK],
                                         _bcast(obc, 1, GBLK))
                fold_range(s, t2, t2f, 0, gs, first, f"{tag}a")
                if pool_n:
                    fold_range(s, t2, t2f, gs, G, first, f"{tag}b")

            # ---------------- Phase 1 + r0 shortcut ----------------
            # phase-1-only pools enter LAST so they release LIFO-cleanly
            actx = ExitStack()
            wpool = actx.enter_context(tc.tile_pool(name="wpool", bufs=1))
            xpool = actx.enter_context(tc.tile_pool(name="xpool", bufs=2))
            ph1ps = actx.enter_context(
                tc.tile_pool(name="ph1ps", bufs=2, space="PSUM"))
            x2 = singles.tile([128, G, BC], bf16)
            w2q = [wpool.tile([128, GQ * ND], bf16, tag=f"w2_{q}",
                              name=f"w2t_{q}")
                   for q in range(4)]
            # sub-0 operands stream in per-cpi (6g) chunks so the first
            # phase-1 matmul can start ~1us in; x2 follows the first chunk
            xq0 = []
            for q in range(4):
                xq = xpool.tile([128, GQ, BS, IG], bf16, tag="xq",
                                name=f"xq0_{q}")
                xq0.append(xq)
            CH = GCP * BS * IG
            CHW = GCP * ND
            for c in range(NCP):
                q, j = c // 3, c % 3
                nc.sync.dma_start(
                    _view(xq0[q], j * CH, [[1, CH]]),
                    xb_d[:, 0, c * CH:(c + 1) * CH])
                nc.sync.dma_start(
                    _view(w2q[q], j * CHW, [[1, CHW]]),
                    w2_d[:, c * CHW:(c + 1) * CHW])
                if c == 0:
                    nc.sync.dma_start(x2, x2_d[:])
            bm32 = singles.tile([128, NSUB, NSUB * BS], bf16)
            nc.sync.dma_start(bm32, bm32_d[:])
            bc32 = singles.tile([32, NSUB, 128], bf16)
            nc.sync.dma_start(bc32, bc32_d[:])
            po0 = outps.tile([32, NSUB * PREG], f32, tag="po", name="po0")
            obc0 = {}

            for s in range(NSUB):
                if s == 0:
                    xqs = xq0
                else:
                    xqs = []
                    for q in range(4):
                        xq = xpool.tile([128, GQ, BS, IG], bf16,
                                        tag="xq", name=f"xq{s}_{q}")
                        nc.sync.dma_start(
                            xq, xb_d[:, s, q * GQ * BS * IG:
                                     (q + 1) * GQ * BS * IG])
                        xqs.append(xq)
                U = upool.tile([128, G, D, N], bf16, tag="U")
                Us.append(U)
                for cpi in range(NCP):
                    ps = ph1ps.tile([128, 1024], f32, tag="ph1")
                    for j in range(GCP):
                        g = cpi * GCP + j
                        q, gq = g // GQ, g % GQ
                        nc.tensor.matmul(
                            _view(ps, (j // 3) * 512 + (j % 3) * ND,
                                  [[1, ND]]),
                            xqs[q][:, gq],
                            w2q[q][:, gq * ND:(gq + 1) * ND],
                            start=True, stop=True)
                    # r0 shortcut matmuls paced through sub 0's loop
                    if s == 0:
                        for g in range(cpi * GCP, (cpi + 1) * GCP):
                            q, gq = g // GQ, g % GQ
                            nc.tensor.matmul(
                                _view(po0, 0, [[1, ND]]),
                                x2[:, g],
                                w2q[q][:, gq * ND:(gq + 1) * ND],
                                start=(g == 0), stop=(g == G - 1),
                                skip_group_check=True)
                    ucopy(s * NCP + cpi,
                          _view(U, cpi * GCP * ND,
                                [[3 * ND, 2], [ND, 3], [1, ND]]),
                          _view(ps, 0, [[512, 2], [ND, 3], [1, ND]]))
                if s == 0:
                    # r0 squash (all 32 rows at once) + per-sub broadcasts
                    ov0 = squash(po0, 0, 0, None, 1.0 / N)
                    for s2 in range(NSUB):
                        obc0[s2] = bcast_out(ov0, 0, s2, eng_d=False)
                else:
                    # agreement r0 for the previous sub overlaps this sub
                    agr_sub(s - 1, obc0[s - 1], first=True, tag=f"a0{s - 1}",
                            pool_n=AGR0_PN[s - 1])
            agr_sub(NSUB - 1, obc0[NSUB - 1], first=True, tag="a03",
                    pool_n=AGR0_PN[NSUB - 1])
            actx.close()
            if DBG:
                dv = tiny.tile([32, ND], f32, tag="dbgv")
                nc.vector.tensor_copy(dv, _view(po0, 0, [[1, ND]]))
                nc.sync.dma_start(dbg_v0[:], dv)
                nc.sync.dma_start(dbg_bl[:], _view(bl, 0, [[1, NSUB * G * N]]))
                nc.sync.dma_start(dbg_u[:], _view(Us[0], 0, [[1, G * ND]]))

            # ---------------- r1, r2 (streamed per sub) ----------------
            es = smpool.tile([128, NSUB, G, N], bf16, tag="e")
            zs = smpool.tile([128, NSUB, G], f32, tag="z")
            rzb = smpool.tile([128, NSUB, G], bf16, tag="rzb")

            for r in (1, 2):
                po = outps.tile([32, NSUB * PREG], f32, tag="po",
                                name=f"po{r}")
                def finish(s):
                    if r == 1:
                        ov = squash(po, s * PREG, r, s, 1.0)
                        obc = bcast_out(ov, r, s, eng_d=False)
                        agr_sub(s, obc, first=False, tag=f"a1{s}",
                                pool_n=AGR1_PN[s])
                    else:
                        squash(po, s * PREG, r, s, 1.0, emit_y=True)

                for s in range(NSUB):
                    nc.scalar.activation(es[:, s], bl[:, s], AF.Exp)
                    nc.vector.tensor_reduce(
                        zs[:, s], es[:, s], axis=AX.X, op=OP.add)
                    nc.vector.reciprocal(zs[:, s], zs[:, s])
                    nc.scalar.copy(rzb[:, s], zs[:, s])
                    # cs = es * (1/z), written in place over es
                    nc.vector.tensor_mul(
                        es[:, s], es[:, s], _bcast(rzb[:, s], 2, N))
                    pn = TM_PN[r][s]
                    gs = (NBLK - pn) * GBLK
                    tm = tpool.tile([128, G, D, N], bf16, tag="t2",
                                    name=f"tm{r}_{s}")
                    nc.vector.tensor_mul(
                        tm[:, 0:gs], Us[s][:, 0:gs],
                        _bcast(es[:, s, 0:gs], 2, D))
                    for blk in range(NBLK - pn, NBLK):
                        g0 = blk * GBLK
                        nc.gpsimd.tensor_mul(
                            tm[:, g0:g0 + GBLK], Us[s][:, g0:g0 + GBLK],
                            _bcast(es[:, s, g0:g0 + GBLK], 2, D))
                    # previous sub's squash/obc lands between po chains on
                    # the PE queue so its agreement isn't stuck behind all
                    # four po chains
                    if s > 0:
                        finish(s - 1)
                    for g in range(G):
                        nc.tensor.matmul(
                            _view(po, s * PREG, [[1, ND]]),
                            bm32[:, s], tm[:, g],
                            start=(g == 0), stop=(g == G - 1),
                            skip_group_check=True)
                finish(NSUB - 1)
    _legalize_waits(nc)
    return nc


def _prep_inputs(inputs, W):
    """Host-side layout prep. Returns per-core input maps."""
    W = np.asarray(W, dtype=np.float32)
    inputs = np.asarray(inputs, dtype=np.float32)
    # W2[(ig,k), (g,d,n)] = W[n, 16g+ig, d, k]
    Wr = W.reshape(N, G, IG, D, DK)
    w2 = np.ascontiguousarray(
        Wr.transpose(2, 4, 1, 3, 0)).reshape(128, G * ND).astype(BF16)
    # bmask32[(b,ig), s, 8s'+b'] = (b==b')(s==s')
    bm32 = np.zeros((BS, IG, NSUB, NSUB, BS), np.float32)
    for s in range(NSUB):
        for b in range(BS):
            bm32[b, :, s, s, b] = 1.0
    bm32 = bm32.reshape(128, NSUB, NSUB * BS).astype(BF16)
    # bcmask32[8s'+b', s, (b,ig)] = (b==b')(s==s')
    bc32 = np.zeros((NSUB, BS, NSUB, BS, IG), np.float32)
    for s in range(NSUB):
        for b in range(BS):
            bc32[s, b, s, b, :] = 1.0
    bc32 = bc32.reshape(32, NSUB, 128).astype(BF16)

    in_maps = []
    for cc in range(NCORES):
        xcore = inputs[cc * BC:(cc + 1) * BC]       # [32, 1152, 8]
        xr = xcore.reshape(NSUB, BS, G, IG, DK)     # [s, b, g, ig, k]
        # zero-padded block-diagonal stationary:
        # xq[(ig,k), s, (g, b, ig')] = x[s*8+b, 16g+ig, k] * (ig==ig')
        xq = np.zeros((IG, DK, NSUB, G, BS, IG), np.float32)
        for ig in range(IG):
            xq[ig, :, :, :, :, ig] = xr[:, :, :, ig, :].transpose(3, 0, 2, 1)
        xq = xq.reshape(128, NSUB, G * BS * IG).astype(BF16)
        # x2[(ig,k), (g, s*8+b)] = x[s*8+b, 16g+ig, k]  (dense)
        x2 = np.ascontiguousarray(
            xr.transpose(3, 4, 2, 0, 1)).reshape(128, G * BC).astype(BF16)
        in_maps.append({"xblk": xq, "x2": x2, "w2": w2,
                        "bmask32": bm32, "bcmask32": bc32})
    return in_maps


def _run(inputs, W, trace=False):
    from concourse.bass_utils import run_bass_kernel_spmd

    if "nc" not in _cache:
        _cache["nc"] = _build_nc()
    nc = _cache["nc"]
    in_maps = _prep_inputs(inputs, W)
    res = run_bass_kernel_spmd(
        nc, in_maps, core_ids=list(range(NCORES)), trace=trace)
    # y[s, (s,b), (d, n)] per core -> out[b_global, n, d]
    out = np.empty((B, N, D), np.float32)
    for cc in range(NCORES):
        yc = res.results[cc]["y"].reshape(NSUB, NSUB * BS, D, N)
        for s in range(NSUB):
            blk = yc[s, s * BS:(s + 1) * BS]        # [8, D, N]
            out[cc * BC + s * BS:cc * BC + (s + 1) * BS] = \
                blk.transpose(0, 2, 1)
    return out, res


def kernel(inputs, W):
    out, _ = _run(inputs, W, trace=False)
    return out
